# revision 5
# baseline (speedup 1.0000x reference)
"""Trainium2 Bass kernel for nn_K_Rectify (gnn message passing, idw + rmsnorm).

Reference computation (B=128, NTOT=129, N=128, GS=16, C=384):
    x   = f[:, 1:, :]                         # [B, N, C]
    nf  = x.reshape(B*N, C)[idx]              # [B, N, GS, C] gather (global flat idx)
    w   = 1/(dist+eps); w /= w.sum(-1)        # idw weights
    sf  = sum_g w * (nf - x) = (sum_g w*nf) - x    (weights sum to 1)
    out = (rf[1:] + x) + rmsnorm(sf) * knorm_w
    cat cls token back on.

Sharding: data-parallel over batch B across 8 cores (16 batches / core).
idx values index the full flattened [B*N] table, so the gather source
table is replicated to every core.

v3 design (bottleneck in v2 was the SWDGE gather path — Q7 descriptor
generation + DMA transfer of 25 MB/core):
  * hybrid-precision gather: per point, the 2 highest-idw-weight
    neighbors come from a bf16 table (768 B rows), the other 14 from an
    fp8e4m3 table (384 B payload out of 512 B-strided rows) -> 44% less
    DMA traffic at ~baseline accuracy (the error is dominated by the
    large-weight neighbors, which stay bf16).
  * all 16 tiles' neighbor buffers stay resident in SBUF, so gathers
    never stall on buffer reuse (v2 serialized on a 5-deep pool).
  * all 16 neighbor groups are summed on the TensorEngine via diag
    matmuls (bf16 dmat x fp8/bf16 rhs mixed-dtype), plus one -I matmul
    that subtracts the center x inside PSUM.  The DVE/ACT product path
    from v2 is gone.
  * per-tile diag-weight build is split DVE/GpSimd; rmsnorm tail is
    Square+accum (ACT), Rsqrt (ACT), and one fused (acc*rr)+fb
    scalar_tensor_tensor (DVE).
"""

import sys

sys.path.insert(0, "/opt/trn_rl_repo")

import ml_dtypes
import numpy as np

import concourse.bacc as bacc
import concourse.mybir as mybir
import concourse.tile as tile
from concourse.bass_utils import run_bass_kernel_spmd

B, NTOT, N, GS, C = 128, 129, 128, 16, 384
EPS = 0.05
RMS_EPS = 1e-6
NCORES = 8
SHB = B // NCORES            # batches per core (16)
PTS = SHB * N                # points per core (2048)
P = 128                      # partitions
TILES = PTS // P             # point-tiles per core (16); tile j == batch j
ROWS = B * N                 # gather table rows (16384)

NBF = 2                      # top-weight neighbor groups gathered in bf16
NFP = GS - NBF               # groups gathered in fp8 (14)
FP8_STRIDE = 512             # fp8 table row stride bytes (256-multiple req'd)
FP8_ELEM = 384               # fp8 payload bytes actually gathered per row

F32 = mybir.dt.float32
BF16 = mybir.dt.bfloat16
FP8 = mybir.dt.float8e4
I16 = mybir.dt.int16

_CACHE = {}


def _dma_gather_fp8(ns, out_ap, in_ap, idxs_ap, num_idxs, queue_num,
                    num_idxs_reg=None, single_packet=True):
    """dma_gather of FP8_ELEM-byte payloads from FP8_STRIDE-strided rows.

    Mirrors bass.EngineNamespace.dma_gather (non-transpose, HBM source)
    except for the `elem_size_bytes % 256 == 0` assert, which the SWDGE
    ucode only requires in transpose mode; the HBM row stride (which IS
    encoded in 256 B units) stays 512.
    """
    _in_ap = ns.lower_ap_dma(in_ap, for_custom_bir_dma=True)
    _idxs_ap = ns.lower_ap(idxs_ap)
    _out_ap = ns.lower_ap(out_ap)
    return ns.add_instruction(
        mybir.InstDMAGatherAnt(
            name=ns.bass.get_next_instruction_name(),
            ins=[*_in_ap, _idxs_ap, ns.lower_val_access(
                ns.to_reg(num_idxs if num_idxs_reg is None else num_idxs_reg))],
            outs=[_out_ap],
            transpose=False,
            num_idxs=num_idxs,
            elem_size=FP8_ELEM,
            stride_bytes_256=FP8_STRIDE // 256,
            gen_mode=0,
            single_packet=single_packet,
            queue_num=queue_num,
            sbuf_tokens_per_rank=0,
            sbuf_free_dim_per_rank=0,
            sbuf_free_dim_pad_per_rank=0,
            sbuf_byte_offset=0,
        )
    )


def _build(knw_is_ones=True):
    nc = bacc.Bacc(
        "TRN2", target_bir_lowering=False, debug=False,
        dynamic_dma_scratch_size=16384, num_swdge_queues=4,
    )

    xall16 = nc.dram_tensor("xall16", [ROWS, C], BF16, kind="ExternalInput")
    xall8 = nc.dram_tensor("xall8", [ROWS, FP8_STRIDE], FP8,
                           kind="ExternalInput")
    xs = nc.dram_tensor("xs", [PTS, C], BF16, kind="ExternalInput")
    idx16_d = nc.dram_tensor("idx16", [P, TILES * NBF * P // 16], I16,
                             kind="ExternalInput")
    idx8_d = nc.dram_tensor("idx8", [P, TILES * NFP * P // 16], I16,
                            kind="ExternalInput")
    wnb_d = nc.dram_tensor("wnb", [P, TILES * GS], BF16, kind="ExternalInput")
    rfx = nc.dram_tensor("rfx", [P, C], BF16, kind="ExternalInput")
    knw = nc.dram_tensor("knw", [P, C], BF16, kind="ExternalInput")
    identd = nc.dram_tensor("identd", [P, GS * P], BF16, kind="ExternalInput")
    negid_d = nc.dram_tensor("negid", [P, P], BF16, kind="ExternalInput")
    out = nc.dram_tensor("out", [PTS, C], BF16, kind="ExternalOutput")

    with tile.TileContext(nc) as tc:
        with (
            tc.tile_pool(name="consts", bufs=1) as cpool,
            tc.tile_pool(name="work", bufs=3) as wpool,
            tc.tile_pool(name="small", bufs=8) as spool,
            tc.tile_pool(name="outs", bufs=4) as opool,
            tc.tile_pool(name="psum", bufs=5, space="PSUM") as ppool,
        ):
            # ---- preamble: idx first, warm every queue-pair -----------
            # Gather index lists come first (they gate everything); the Q7
            # ucode library load (~7-10us) is absorbed by one tiny dummy
            # gather per SWDGE queue so the 4 Q7 pairs all warm in parallel.
            idx16_t = cpool.tile([P, TILES * NBF * P // 16], I16)
            nc.sync.dma_start(idx16_t[:], idx16_d[:])
            idx8_t = cpool.tile([P, TILES * NFP * P // 16], I16)
            nc.sync.dma_start(idx8_t[:], idx8_d[:])
            dummy_idx = cpool.tile([P, 8], I16)
            nc.vector.memset(dummy_idx[:], 0)
            dummy_out = cpool.tile([P, 1, C], BF16)
            regP = nc.gpsimd.to_reg(P)
            reg1024 = nc.gpsimd.to_reg(4 * NBF * P)
            reg896 = nc.gpsimd.to_reg(7 * P)
            for q in range(4):
                nc.gpsimd.dma_gather(
                    out_ap=dummy_out[:], in_ap=xall16[:], idxs_ap=dummy_idx[:],
                    num_idxs=P, num_idxs_reg=regP, elem_size=C, queue_num=q,
                )

            wnb_t = cpool.tile([P, TILES * GS], BF16)
            nc.scalar.dma_start(wnb_t[:], wnb_d[:])
            rfx_t = cpool.tile([P, C], BF16)
            nc.scalar.dma_start(rfx_t[:], rfx[:])
            identr = cpool.tile([P, GS, P], BF16)
            nc.scalar.dma_start(identr[:], identd[:].rearrange(
                "p (g q) -> p g q", g=GS))
            negid_t = cpool.tile([P, P], BF16)
            nc.scalar.dma_start(negid_t[:], negid_d[:])
            if not knw_is_ones:
                knw_t = cpool.tile([P, C], BF16)
                nc.scalar.dma_start(knw_t[:], knw[:])
            xt_all = cpool.tile([P, TILES, C], BF16)
            nc.scalar.dma_start(
                xt_all[:], xs[:].rearrange("(j p) c -> p j c", p=P)
            )
            epsb = cpool.tile([P, 1], F32)
            nc.vector.memset(epsb[:], RMS_EPS)

            # fb = x + rfx for all 16 tiles in one shot
            fb_all = cpool.tile([P, TILES, C], BF16)
            nc.vector.tensor_tensor(
                out=fb_all[:],
                in0=xt_all[:],
                in1=rfx_t[:].rearrange("p (x c) -> p x c", x=1).to_broadcast(
                    [P, TILES, C]
                ),
                op=mybir.AluOpType.add,
            )

            # ---- resident neighbor buffers ----------------------------
            # bf16 top-2 groups: [P, TILES*NBF, C]; fp8 rest: [P, TILES*NFP, C]
            nbr16 = cpool.tile([P, TILES * NBF, C], BF16)
            nbr8 = cpool.tile([P, TILES * NFP, C], FP8)

            # ---- gathers: all issued up front, 4 queues ---------------
            qi = 0
            S16 = NBF * 4 * P // 16      # idx cols per 4-tile bf16 batch (64)
            S8 = 7 * P // 16             # idx cols per 7-group fp8 gather (56)
            for j in range(TILES):
                if j % 4 == 0:
                    t = j // 4
                    nc.gpsimd.dma_gather(
                        out_ap=nbr16[:, t * 4 * NBF : (t + 1) * 4 * NBF, :],
                        in_ap=xall16[:],
                        idxs_ap=idx16_t[:, t * S16 : (t + 1) * S16],
                        num_idxs=4 * NBF * P,
                        num_idxs_reg=reg1024,
                        elem_size=C,
                        queue_num=qi % 4,
                    )
                    qi += 1
                for h in range(2):
                    _dma_gather_fp8(
                        nc.gpsimd,
                        out_ap=nbr8[:, j * NFP + 7 * h : j * NFP + 7 * (h + 1), :],
                        in_ap=xall8[:, :FP8_ELEM],
                        idxs_ap=idx8_t[:, (2 * j + h) * S8 : (2 * j + h + 1) * S8],
                        num_idxs=7 * P,
                        num_idxs_reg=reg896,
                        queue_num=qi % 4,
                    )
                    qi += 1

            # ---- per-tile compute -------------------------------------
            for j in range(TILES):
                wcol = j * GS

                # diag weight matrices for all 16 groups in one DVE op
                # (Pool is saturated dispatching SWDGE gathers).
                dmat = wpool.tile([P, GS, P], BF16, tag="dmat")
                nc.vector.tensor_tensor(
                    out=dmat[:],
                    in0=identr[:],
                    in1=wnb_t[:, wcol : wcol + GS].to_broadcast([P, GS, P]),
                    op=mybir.AluOpType.mult,
                )

                # PSUM: sf = sum_g diag(w_g) @ nbr_g  -  I @ x
                acc_p = ppool.tile([P, C], F32, tag="acc")
                for g in range(NBF):
                    nc.tensor.matmul(
                        out=acc_p[:],
                        lhsT=dmat[:, g, :],
                        rhs=nbr16[:, j * NBF + g, :],
                        start=(g == 0),
                        stop=False,
                    )
                for g in range(NFP):
                    nc.tensor.matmul(
                        out=acc_p[:],
                        lhsT=dmat[:, NBF + g, :],
                        rhs=nbr8[:, j * NFP + g, :],
                        start=False,
                        stop=False,
                    )
                nc.tensor.matmul(
                    out=acc_p[:], lhsT=negid_t[:], rhs=xt_all[:, j, :],
                    start=False, stop=True,
                )

                # rmsnorm tail: ssq on ACT, Rsqrt on ACT, fused scale+add DVE
                ssq = spool.tile([P, 1], F32, tag="ssq")
                sq = wpool.tile([P, C], BF16, tag="sq")
                nc.scalar.activation(
                    out=sq[:], in_=acc_p[:],
                    func=mybir.ActivationFunctionType.Square,
                    accum_out=ssq[:],
                )
                rms = spool.tile([P, 1], F32, tag="rms")
                nc.scalar.activation(
                    out=rms[:], in_=ssq[:],
                    func=mybir.ActivationFunctionType.Sqrt,
                    scale=1.0 / C, bias=epsb[:, :1],
                )
                rr = spool.tile([P, 1], F32, tag="rr")
                nc.vector.reciprocal(rr[:], rms[:])

                if j % 4 == 0:
                    ob4 = opool.tile([P, 4, C], BF16, tag="ob")
                obj = ob4[:, j % 4, :]
                if knw_is_ones:
                    nc.vector.scalar_tensor_tensor(
                        out=obj, in0=acc_p[:], scalar=rr[:, :1],
                        in1=fb_all[:, j, :],
                        op0=mybir.AluOpType.mult, op1=mybir.AluOpType.add,
                    )
                else:
                    nt = wpool.tile([P, C], BF16, tag="nt")
                    nc.scalar.activation(
                        out=nt[:], in_=acc_p[:],
                        func=mybir.ActivationFunctionType.Copy,
                        scale=rr[:, :1],
                    )
                    nc.vector.tensor_tensor(
                        out=nt[:], in0=nt[:], in1=knw_t[:],
                        op=mybir.AluOpType.mult,
                    )
                    nc.vector.tensor_tensor(
                        out=obj, in0=fb_all[:, j, :], in1=nt[:],
                        op=mybir.AluOpType.add,
                    )

                if j % 4 == 3:
                    t4 = j // 4
                    nc.sync.dma_start(
                        out[t4 * 4 * P : (t4 + 1) * 4 * P, :].rearrange(
                            "(a p) c -> p a c", p=P),
                        ob4[:],
                    )

    nc.compile()
    return nc


def _get_nc(knw_is_ones=True):
    key = ("nc3", knw_is_ones)
    if key not in _CACHE:
        _CACHE[key] = _build(knw_is_ones)
    return _CACHE[key]


def _wrap(lst):
    """Gather list (len n, mult of 16) -> [P, n/16] i16 wrapped for SWDGE."""
    n = len(lst)
    wrapped = np.asarray(lst, np.int16).reshape(n // 16, 16).T
    return np.tile(wrapped, (P // 16, 1))


def _make_in_maps(inputs):
    f = np.asarray(inputs["f"], dtype=np.float32)
    distance = np.asarray(inputs["distance"], dtype=np.float32)
    rf = np.asarray(inputs["rf"], dtype=np.float32)
    knorm_w = np.asarray(inputs["knorm_w"], dtype=np.float32)
    idx_np = np.asarray(inputs["idx"]).astype(np.int64)

    x = np.ascontiguousarray(f[:, NTOT - N :, :].reshape(ROWS, C))
    x_bf = np.ascontiguousarray(x.astype(ml_dtypes.bfloat16))
    x_f8 = np.zeros((ROWS, FP8_STRIDE), ml_dtypes.float8_e4m3fn)
    x_f8[:, :C] = x.astype(ml_dtypes.float8_e4m3fn)
    rfx_np = np.ascontiguousarray(rf[NTOT - N :][:P].astype(ml_dtypes.bfloat16))
    ident16 = np.ascontiguousarray(
        np.tile(np.eye(P, dtype=ml_dtypes.bfloat16), (1, GS))
    )
    negid = np.ascontiguousarray((-np.eye(P)).astype(ml_dtypes.bfloat16))
    knw_np = np.ascontiguousarray(
        np.broadcast_to(knorm_w, (P, C)).astype(ml_dtypes.bfloat16)
    )

    # normalized idw weights; per-point descending-weight group order so
    # groups 0..1 (bf16-gathered) carry the most weight
    w = 1.0 / (distance + EPS)
    w = w / w.sum(axis=-1, keepdims=True)            # [B, N, GS] f32
    order = np.argsort(-w, axis=-1)
    w_s = np.take_along_axis(w, order, axis=-1)
    idx_s = np.take_along_axis(idx_np, order, axis=-1)

    in_maps = []
    for c in range(NCORES):
        bs = slice(c * SHB, (c + 1) * SHB)
        a = idx_s[bs].reshape(TILES, P, GS)          # [j, p, g]
        wc = w_s[bs].reshape(TILES, P, GS)

        # bf16 gathers: 4 batches x 1024 idx, order (j_local, g, p)
        cols16 = []
        for t in range(TILES // 4):
            lst = a[4 * t : 4 * t + 4, :, :NBF].transpose(0, 2, 1).reshape(-1)
            cols16.append(_wrap(lst))
        idx16 = np.concatenate(cols16, axis=1)

        # fp8 gathers: per tile 2 x 896 idx (7 groups), order (g, p)
        cols8 = []
        for j in range(TILES):
            for h in range(2):
                gsl = slice(NBF + 7 * h, NBF + 7 * (h + 1))
                lst = a[j, :, gsl].T.reshape(-1)
                cols8.append(_wrap(lst))
        idx8 = np.concatenate(cols8, axis=1)

        wn_core = (
            wc.transpose(1, 0, 2).reshape(P, TILES * GS)
        )
        in_maps.append(
            {
                "xall16": x_bf,
                "xall8": x_f8,
                "xs": np.ascontiguousarray(x_bf[c * PTS : (c + 1) * PTS]),
                "idx16": np.ascontiguousarray(idx16),
                "idx8": np.ascontiguousarray(idx8),
                "wnb": np.ascontiguousarray(
                    wn_core.astype(ml_dtypes.bfloat16)),
                "rfx": rfx_np,
                "knw": knw_np,
                "identd": ident16,
                "negid": negid,
            }
        )
    return in_maps


def kernel(f, distance, rf, knorm_w, idx, **_unused):
    f = np.ascontiguousarray(np.asarray(f, dtype=np.float32))
    in_maps = _make_in_maps(
        {"f": f, "distance": distance, "rf": rf, "knorm_w": knorm_w, "idx": idx}
    )

    nc = _get_nc(bool(np.all(np.asarray(knorm_w) == 1.0)))
    res = run_bass_kernel_spmd(nc, in_maps, list(range(NCORES)))

    out = np.empty((B, NTOT, C), np.float32)
    out[:, : NTOT - N, :] = f[:, : NTOT - N, :]
    body = np.concatenate(
        [np.asarray(res.results[c]["out"]).astype(np.float32)
         for c in range(NCORES)],
        axis=0,
    )
    out[:, NTOT - N :, :] = body.reshape(B, N, C)
    return out


# revision 7
# speedup vs baseline: 1.0787x; 1.0787x over previous
"""Trainium2 Bass kernel for nn_K_Rectify (gnn message passing, idw + rmsnorm).

Reference computation (B=128, NTOT=129, N=128, GS=16, C=384):
    x   = f[:, 1:, :]                         # [B, N, C]
    nf  = x.reshape(B*N, C)[idx]              # [B, N, GS, C] gather (global flat idx)
    w   = 1/(dist+eps); w /= w.sum(-1)        # idw weights
    sf  = sum_g w * (nf - x) = (sum_g w*nf) - x    (weights sum to 1)
    out = (rf[1:] + x) + rmsnorm(sf) * knorm_w
    cat cls token back on.

Sharding: data-parallel over batch B across 8 cores (16 batches / core).
idx values index the full flattened [B*N] table, so the gather source
table is replicated to every core.

v3 design (bottleneck in v2 was the SWDGE gather path — Q7 descriptor
generation + DMA transfer of 25 MB/core):
  * hybrid-precision gather: per point, the 2 highest-idw-weight
    neighbors come from a bf16 table (768 B rows), the other 14 from an
    fp8e4m3 table (384 B payload out of 512 B-strided rows) -> 44% less
    DMA traffic at ~baseline accuracy (the error is dominated by the
    large-weight neighbors, which stay bf16).
  * all 16 tiles' neighbor buffers stay resident in SBUF, so gathers
    never stall on buffer reuse (v2 serialized on a 5-deep pool).
  * all 16 neighbor groups are summed on the TensorEngine via diag
    matmuls (bf16 dmat x fp8/bf16 rhs mixed-dtype), plus one -I matmul
    that subtracts the center x inside PSUM.  The DVE/ACT product path
    from v2 is gone.
  * per-tile diag-weight build is split DVE/GpSimd; rmsnorm tail is
    Square+accum (ACT), Rsqrt (ACT), and one fused (acc*rr)+fb
    scalar_tensor_tensor (DVE).
"""

import sys

sys.path.insert(0, "/opt/trn_rl_repo")

import ml_dtypes
import numpy as np

import concourse.bacc as bacc
import concourse.mybir as mybir
import concourse.tile as tile
from concourse.bass_utils import run_bass_kernel_spmd

B, NTOT, N, GS, C = 128, 129, 128, 16, 384
EPS = 0.05
RMS_EPS = 1e-6
NCORES = 8
SHB = B // NCORES            # batches per core (16)
PTS = SHB * N                # points per core (2048)
P = 128                      # partitions
TILES = PTS // P             # point-tiles per core (16); tile j == batch j
ROWS = B * N                 # gather table rows (16384)

NBF = 2                      # top-weight neighbor groups gathered in bf16
NFP = GS - NBF               # groups gathered in fp8 (14)
FP8_STRIDE = 512             # fp8 table row stride bytes (256-multiple req'd)
FP8_ELEM = 384               # fp8 payload bytes actually gathered per row

F32 = mybir.dt.float32
BF16 = mybir.dt.bfloat16
FP8 = mybir.dt.float8e4
I16 = mybir.dt.int16

_CACHE = {}


def _dma_gather_fp8(ns, out_ap, in_ap, idxs_ap, num_idxs, queue_num,
                    num_idxs_reg=None, single_packet=True):
    """dma_gather of FP8_ELEM-byte payloads from FP8_STRIDE-strided rows.

    Mirrors bass.EngineNamespace.dma_gather (non-transpose, HBM source)
    except for the `elem_size_bytes % 256 == 0` assert, which the SWDGE
    ucode only requires in transpose mode; the HBM row stride (which IS
    encoded in 256 B units) stays 512.
    """
    _in_ap = ns.lower_ap_dma(in_ap, for_custom_bir_dma=True)
    _idxs_ap = ns.lower_ap(idxs_ap)
    _out_ap = ns.lower_ap(out_ap)
    return ns.add_instruction(
        mybir.InstDMAGatherAnt(
            name=ns.bass.get_next_instruction_name(),
            ins=[*_in_ap, _idxs_ap, ns.lower_val_access(
                ns.to_reg(num_idxs if num_idxs_reg is None else num_idxs_reg))],
            outs=[_out_ap],
            transpose=False,
            num_idxs=num_idxs,
            elem_size=FP8_ELEM,
            stride_bytes_256=FP8_STRIDE // 256,
            gen_mode=0,
            single_packet=single_packet,
            queue_num=queue_num,
            sbuf_tokens_per_rank=0,
            sbuf_free_dim_per_rank=0,
            sbuf_free_dim_pad_per_rank=0,
            sbuf_byte_offset=0,
        )
    )


def _build(knw_is_ones=True):
    nc = bacc.Bacc(
        "TRN2", target_bir_lowering=False, debug=False,
        dynamic_dma_scratch_size=16384, num_swdge_queues=4,
    )

    xall16 = nc.dram_tensor("xall16", [ROWS, C], BF16, kind="ExternalInput")
    xall8 = nc.dram_tensor("xall8", [ROWS, FP8_STRIDE], FP8,
                           kind="ExternalInput")
    xs = nc.dram_tensor("xs", [PTS, C], BF16, kind="ExternalInput")
    idx16_d = nc.dram_tensor("idx16", [P, TILES * NBF * P // 16], I16,
                             kind="ExternalInput")
    idx8_d = nc.dram_tensor("idx8", [P, TILES * NFP * P // 16], I16,
                            kind="ExternalInput")
    wnb_d = nc.dram_tensor("wnb", [P, TILES * GS], BF16, kind="ExternalInput")
    rfx = nc.dram_tensor("rfx", [P, C], BF16, kind="ExternalInput")
    knw = nc.dram_tensor("knw", [P, C], BF16, kind="ExternalInput")
    identd = nc.dram_tensor("identd", [P, GS * P], BF16, kind="ExternalInput")
    negid_d = nc.dram_tensor("negid", [P, P], BF16, kind="ExternalInput")
    out = nc.dram_tensor("out", [PTS, C], BF16, kind="ExternalOutput")

    with tile.TileContext(nc) as tc:
        with (
            tc.tile_pool(name="consts", bufs=1) as cpool,
            tc.tile_pool(name="work", bufs=3) as wpool,
            tc.tile_pool(name="small", bufs=8) as spool,
            tc.tile_pool(name="outs", bufs=4) as opool,
            tc.tile_pool(name="psum", bufs=5, space="PSUM") as ppool,
        ):
            # ---- preamble: idx first, warm every queue-pair -----------
            # Gather index lists come first (they gate everything); the Q7
            # ucode library load (~7-10us) is absorbed by one tiny dummy
            # gather per SWDGE queue so the 4 Q7 pairs all warm in parallel.
            idx16_t = cpool.tile([P, TILES * NBF * P // 16], I16)
            nc.sync.dma_start(idx16_t[:], idx16_d[:])
            idx8_t = cpool.tile([P, TILES * NFP * P // 16], I16)
            nc.sync.dma_start(idx8_t[:], idx8_d[:])
            dummy_idx = cpool.tile([P, 8], I16)
            nc.vector.memset(dummy_idx[:], 0)
            dummy_out = cpool.tile([P, 1, C], BF16)
            regP = nc.gpsimd.to_reg(P)
            reg1024 = nc.gpsimd.to_reg(4 * NBF * P)
            reg896 = nc.gpsimd.to_reg(7 * P)
            nc.gpsimd.dma_gather(
                out_ap=dummy_out[:], in_ap=xall16[:], idxs_ap=dummy_idx[:],
                num_idxs=P, num_idxs_reg=regP, elem_size=C, queue_num=0,
            )

            wnb_t = cpool.tile([P, TILES * GS], BF16)
            nc.scalar.dma_start(wnb_t[:], wnb_d[:])
            rfx_t = cpool.tile([P, C], BF16)
            nc.scalar.dma_start(rfx_t[:], rfx[:])
            identr = cpool.tile([P, GS, P], BF16)
            nc.scalar.dma_start(identr[:], identd[:].rearrange(
                "p (g q) -> p g q", g=GS))
            negid_t = cpool.tile([P, P], BF16)
            nc.scalar.dma_start(negid_t[:], negid_d[:])
            if not knw_is_ones:
                knw_t = cpool.tile([P, C], BF16)
                nc.scalar.dma_start(knw_t[:], knw[:])
            xt_all = cpool.tile([P, TILES, C], BF16)
            nc.scalar.dma_start(
                xt_all[:], xs[:].rearrange("(j p) c -> p j c", p=P)
            )
            epsb = cpool.tile([P, 1], F32)
            nc.vector.memset(epsb[:], RMS_EPS)

            # fb = x + rfx for all 16 tiles in one shot
            fb_all = cpool.tile([P, TILES, C], BF16)
            nc.vector.tensor_tensor(
                out=fb_all[:],
                in0=xt_all[:],
                in1=rfx_t[:].rearrange("p (x c) -> p x c", x=1).to_broadcast(
                    [P, TILES, C]
                ),
                op=mybir.AluOpType.add,
            )

            sq_scr = cpool.tile([P, C], BF16)

            # ---- resident neighbor buffers ----------------------------
            # bf16 top-2 groups: [P, TILES*NBF, C]; fp8 rest: [P, TILES*NFP, C]
            nbr16 = cpool.tile([P, TILES * NBF, C], BF16)
            nbr8 = cpool.tile([P, TILES * NFP, C], FP8)

            # ---- gathers: all issued up front, 4 queues ---------------
            # start on q1: q0's pair absorbs the ucode library load behind
            # the dummy and gets its first real gather last
            qi = 1
            S16 = NBF * 4 * P // 16      # idx cols per 4-tile bf16 batch (64)
            S8 = 7 * P // 16             # idx cols per 7-group fp8 gather (56)
            for j in range(TILES):
                if j % 4 == 0:
                    t = j // 4
                    nc.gpsimd.dma_gather(
                        out_ap=nbr16[:, t * 4 * NBF : (t + 1) * 4 * NBF, :],
                        in_ap=xall16[:],
                        idxs_ap=idx16_t[:, t * S16 : (t + 1) * S16],
                        num_idxs=4 * NBF * P,
                        num_idxs_reg=reg1024,
                        elem_size=C,
                        queue_num=qi % 4,
                    )
                    qi += 1
                for h in range(2):
                    _dma_gather_fp8(
                        nc.gpsimd,
                        out_ap=nbr8[:, j * NFP + 7 * h : j * NFP + 7 * (h + 1), :],
                        in_ap=xall8[:, :FP8_ELEM],
                        idxs_ap=idx8_t[:, (2 * j + h) * S8 : (2 * j + h + 1) * S8],
                        num_idxs=7 * P,
                        num_idxs_reg=reg896,
                        queue_num=qi % 4,
                    )
                    qi += 1

            # ---- per-tile compute -------------------------------------
            for j in range(TILES):
                wcol = j * GS

                # diag weight matrices for all 16 groups in one DVE op
                # (Pool is saturated dispatching SWDGE gathers).
                dmat = wpool.tile([P, GS, P], BF16, tag="dmat")
                nc.vector.tensor_tensor(
                    out=dmat[:],
                    in0=identr[:],
                    in1=wnb_t[:, wcol : wcol + GS].to_broadcast([P, GS, P]),
                    op=mybir.AluOpType.mult,
                )

                # PSUM: sf = sum_g diag(w_g) @ nbr_g  -  I @ x
                if j % 4 == 0:
                    acc_keep = []
                acc_p = ppool.tile([P, C], F32, tag="acc")
                acc_keep.append(acc_p)
                for g in range(NBF):
                    nc.tensor.matmul(
                        out=acc_p[:],
                        lhsT=dmat[:, g, :],
                        rhs=nbr16[:, j * NBF + g, :],
                        start=(g == 0),
                        stop=False,
                    )
                for g in range(NFP):
                    nc.tensor.matmul(
                        out=acc_p[:],
                        lhsT=dmat[:, NBF + g, :],
                        rhs=nbr8[:, j * NFP + g, :],
                        start=False,
                        stop=False,
                    )
                nc.tensor.matmul(
                    out=acc_p[:], lhsT=negid_t[:], rhs=xt_all[:, j, :],
                    start=False, stop=True,
                )

                # rmsnorm tail: per-tile Square+accum into a 4-wide ssq;
                # sqrt/reciprocal batched per 4 tiles (fewer instructions
                # and semaphores); fused (acc*rr)+fb on DVE per tile.
                if j % 4 == 0:
                    ssq4 = spool.tile([P, 4], F32, tag="ssq")
                    rms4 = spool.tile([P, 4], F32, tag="rms")
                    rr4 = spool.tile([P, 4], F32, tag="rr")
                    ob4 = opool.tile([P, 4, C], BF16, tag="ob")
                nc.scalar.activation(
                    out=sq_scr[:], in_=acc_p[:],
                    func=mybir.ActivationFunctionType.Square,
                    accum_out=ssq4[:, j % 4 : j % 4 + 1],
                )
                if j % 4 == 3:
                    nc.scalar.activation(
                        out=rms4[:], in_=ssq4[:],
                        func=mybir.ActivationFunctionType.Sqrt,
                        scale=1.0 / C,
                        bias=epsb[:, :1],
                    )
                    nc.vector.reciprocal(rr4[:], rms4[:])
                # defer the output combine of tiles 4t..4t+3 until rr4 ready
                if j % 4 == 3:
                    for m in range(4):
                        jj = j - 3 + m
                        obj = ob4[:, m, :]
                        accm = acc_keep[m]
                        if knw_is_ones:
                            nc.vector.scalar_tensor_tensor(
                                out=obj, in0=accm[:], scalar=rr4[:, m : m + 1],
                                in1=fb_all[:, jj, :],
                                op0=mybir.AluOpType.mult,
                                op1=mybir.AluOpType.add,
                            )
                        else:
                            nt = wpool.tile([P, C], BF16, tag="nt")
                            nc.scalar.activation(
                                out=nt[:], in_=accm[:],
                                func=mybir.ActivationFunctionType.Copy,
                                scale=rr4[:, m : m + 1],
                            )
                            nc.vector.tensor_tensor(
                                out=nt[:], in0=nt[:], in1=knw_t[:],
                                op=mybir.AluOpType.mult,
                            )
                            nc.vector.tensor_tensor(
                                out=obj, in0=fb_all[:, jj, :], in1=nt[:],
                                op=mybir.AluOpType.add,
                            )
                    t4 = j // 4
                    nc.sync.dma_start(
                        out[t4 * 4 * P : (t4 + 1) * 4 * P, :].rearrange(
                            "(a p) c -> p a c", p=P),
                        ob4[:],
                    )

    nc.compile()
    return nc


def _get_nc(knw_is_ones=True):
    key = ("nc3", knw_is_ones)
    if key not in _CACHE:
        _CACHE[key] = _build(knw_is_ones)
    return _CACHE[key]


def _wrap(lst):
    """Gather list (len n, mult of 16) -> [P, n/16] i16 wrapped for SWDGE."""
    n = len(lst)
    wrapped = np.asarray(lst, np.int16).reshape(n // 16, 16).T
    return np.tile(wrapped, (P // 16, 1))


def _make_in_maps(inputs):
    f = np.asarray(inputs["f"], dtype=np.float32)
    distance = np.asarray(inputs["distance"], dtype=np.float32)
    rf = np.asarray(inputs["rf"], dtype=np.float32)
    knorm_w = np.asarray(inputs["knorm_w"], dtype=np.float32)
    idx_np = np.asarray(inputs["idx"]).astype(np.int64)

    x = np.ascontiguousarray(f[:, NTOT - N :, :].reshape(ROWS, C))
    x_bf = np.ascontiguousarray(x.astype(ml_dtypes.bfloat16))
    x_f8 = np.zeros((ROWS, FP8_STRIDE), ml_dtypes.float8_e4m3fn)
    x_f8[:, :C] = x.astype(ml_dtypes.float8_e4m3fn)
    rfx_np = np.ascontiguousarray(rf[NTOT - N :][:P].astype(ml_dtypes.bfloat16))
    ident16 = np.ascontiguousarray(
        np.tile(np.eye(P, dtype=ml_dtypes.bfloat16), (1, GS))
    )
    negid = np.ascontiguousarray((-np.eye(P)).astype(ml_dtypes.bfloat16))
    knw_np = np.ascontiguousarray(
        np.broadcast_to(knorm_w, (P, C)).astype(ml_dtypes.bfloat16)
    )

    # normalized idw weights; per-point descending-weight group order so
    # groups 0..1 (bf16-gathered) carry the most weight
    w = 1.0 / (distance + EPS)
    w = w / w.sum(axis=-1, keepdims=True)            # [B, N, GS] f32
    order = np.argsort(-w, axis=-1)
    w_s = np.take_along_axis(w, order, axis=-1)
    idx_s = np.take_along_axis(idx_np, order, axis=-1)

    in_maps = []
    for c in range(NCORES):
        bs = slice(c * SHB, (c + 1) * SHB)
        a = idx_s[bs].reshape(TILES, P, GS)          # [j, p, g]
        wc = w_s[bs].reshape(TILES, P, GS)

        # bf16 gathers: 4 batches x 1024 idx, order (j_local, g, p)
        cols16 = []
        for t in range(TILES // 4):
            lst = a[4 * t : 4 * t + 4, :, :NBF].transpose(0, 2, 1).reshape(-1)
            cols16.append(_wrap(lst))
        idx16 = np.concatenate(cols16, axis=1)

        # fp8 gathers: per tile 2 x 896 idx (7 groups), order (g, p)
        cols8 = []
        for j in range(TILES):
            for h in range(2):
                gsl = slice(NBF + 7 * h, NBF + 7 * (h + 1))
                lst = a[j, :, gsl].T.reshape(-1)
                cols8.append(_wrap(lst))
        idx8 = np.concatenate(cols8, axis=1)

        wn_core = (
            wc.transpose(1, 0, 2).reshape(P, TILES * GS)
        )
        in_maps.append(
            {
                "xall16": x_bf,
                "xall8": x_f8,
                "xs": np.ascontiguousarray(x_bf[c * PTS : (c + 1) * PTS]),
                "idx16": np.ascontiguousarray(idx16),
                "idx8": np.ascontiguousarray(idx8),
                "wnb": np.ascontiguousarray(
                    wn_core.astype(ml_dtypes.bfloat16)),
                "rfx": rfx_np,
                "knw": knw_np,
                "identd": ident16,
                "negid": negid,
            }
        )
    return in_maps


def kernel(f, distance, rf, knorm_w, idx, **_unused):
    f = np.ascontiguousarray(np.asarray(f, dtype=np.float32))
    in_maps = _make_in_maps(
        {"f": f, "distance": distance, "rf": rf, "knorm_w": knorm_w, "idx": idx}
    )

    nc = _get_nc(bool(np.all(np.asarray(knorm_w) == 1.0)))
    res = run_bass_kernel_spmd(nc, in_maps, list(range(NCORES)))

    out = np.empty((B, NTOT, C), np.float32)
    out[:, : NTOT - N, :] = f[:, : NTOT - N, :]
    body = np.concatenate(
        [np.asarray(res.results[c]["out"]).astype(np.float32)
         for c in range(NCORES)],
        axis=0,
    )
    out[:, NTOT - N :, :] = body.reshape(B, N, C)
    return out
